# revision 8
# baseline (speedup 1.0000x reference)
"""Criss-cross (CCNet) attention kernel for Trainium2, 8 NeuronCores.

Sharding: core c in 0..7 -> batch b = c//2, value-channel half h = c%2.
Each core computes, for its (b, h), the full joint row+column softmax
attention over 256 of the 512 value/output channels.

Design (v2, fp16 pipeline):
  - All matmuls are fp16 (1 cyc/row at any moving size, vs fp32r's 4x
    penalty under 256), with fp32 PSUM accumulation.
  - Energies E = k.q are shifted by a global constant DELTA before exp;
    P = exp(E-DELTA) is stored in bf16 (wide exponent range covers the
    ~33-nat spread of per-row maxima; fp16 cannot).  The shift cancels
    exactly in the final combine since both passes share it.
  - Outputs are UN-normalized:  orow = P^T V  in bf16 (wide exponent
    range), plus the row/col sums S (fp32).  The host computes
    out = (o_row + o_col^T) / (S_row + S_col), which equals the exact
    softmax combine; no on-device division or scaling at all.
  - V lives SBUF-resident in column-major slot layout VRES[j, x*258+c]
    (built by 4 small partition-transposing SBUF->SBUF DMAs per row
    chunk), so the column pass needs NO DRAM staging round-trip.
  - Deep software pipeline: iteration i runs projections(i),
    energies(i-1), aggregation+evac(i-2), so the tensor engine queue
    never waits on scalar/vector evacuations and holds its max p-state.
"""

import numpy as np

import concourse.tile as tile
from concourse import bacc, mybir
from concourse.bass_utils import run_bass_kernel_spmd

B, C, H, W = 4, 512, 128, 128
CQK = C // 8          # 64
CV = C // 2           # 256 v channels per core
HW = H * W
N_CORES = 8
NCH = 32              # row chunks (4 rows each)
NSCH = 16             # col super-chunks (8 cols each)
DELTA = 41.0          # exp shift (max energy on this data ~50.7)
VBW = 258             # v slot width: 256 channels + 2 ones columns

F32 = mybir.dt.float32
F16 = mybir.dt.float16
BF16 = mybir.dt.bfloat16
EXP = mybir.ActivationFunctionType.Exp
COPY = mybir.ActivationFunctionType.Copy

_CACHE = {}


def _build(with_bias):
    nc = bacc.Bacc("TRN2", target_bir_lowering=False, debug=False,
                   num_devices=N_CORES)
    nck = 5 if with_bias else 4    # contraction chunks (last = 2 bias rows)

    xin = nc.dram_tensor("xin", [NCH, 128, 2048], F16,
                         kind="ExternalInput").ap()
    xbias = nc.dram_tensor("xbias", [NCH, 2, 2048], F16,
                           kind="ExternalInput").ap() if with_bias else None
    wqk = nc.dram_tensor("wqk", [C + (2 if with_bias else 0), 128], F16,
                         kind="ExternalInput").ap()
    wv = nc.dram_tensor("wv", [C + (2 if with_bias else 0), CV], F16,
                        kind="ExternalInput").ap()
    negid = nc.dram_tensor("negid", [128, 128], F16,
                           kind="ExternalInput").ap()
    id4 = nc.dram_tensor("id4", [128, 512], F16, kind="ExternalInput").ap()

    # orow laid out [x, y, c]; ocol laid out [y, x, c]  (contiguous writes)
    orow = nc.dram_tensor("orow", [W, H, CV], BF16,
                          kind="ExternalOutput").ap()
    ocol = nc.dram_tensor("ocol", [H, W, CV], BF16,
                          kind="ExternalOutput").ap()
    ssr = nc.dram_tensor("ssr", [128, 128], F32, kind="ExternalOutput").ap()
    ssc = nc.dram_tensor("ssc", [128, 128], F32, kind="ExternalOutput").ap()

    with tile.TileContext(nc) as tc:
        with (
            tc.tile_pool(name="cst", bufs=1) as cst,
            tc.tile_pool(name="xs", bufs=3) as xsp,
            tc.tile_pool(name="p4", bufs=3) as p4p,
            tc.tile_pool(name="p4c", bufs=4) as p4cp,
            tc.tile_pool(name="o16r", bufs=2) as o16rp,
            tc.tile_pool(name="o16c", bufs=2) as o16cp,
            tc.tile_pool(name="psbig", bufs=2, space="PSUM") as psbigp,
            tc.tile_pool(name="psv", bufs=2, space="PSUM") as psvp,
            tc.tile_pool(name="psO", bufs=2, space="PSUM") as psOp,
        ):
            # ---- prefetch x chunks 0,1 before everything ----
            xpre = {}
            for ch0 in range(2):
                xt = xsp.tile([128, 2048], F16, tag="xs", name=f"xpre{ch0}")
                nc.sync.dma_start(xt[:], xin[ch0])
                if with_bias:
                    xbt = xsp.tile([2, 2048], F16, tag="xb",
                                   name=f"xbpre{ch0}")
                    nc.sync.dma_start(xbt[:], xbias[ch0])
                    xpre[ch0] = (xt, xbt)
                else:
                    xpre[ch0] = (xt, None)

            # ---- persistent constants ----
            WQK = cst.tile([128, nck * 128], F16)
            for k in range(nck):
                rows = 128 if k < 4 else 2
                nc.sync.dma_start(WQK[0:rows, k * 128:(k + 1) * 128],
                                  wqk[k * 128:k * 128 + rows, :])
            WV = cst.tile([128, nck * CV], F16)
            for k in range(nck):
                rows = 128 if k < 4 else 2
                nc.sync.dma_start(WV[0:rows, k * CV:(k + 1) * CV],
                                  wv[k * 128:k * 128 + rows, :])
            NEGID = cst.tile([128, 128], F16)
            nc.sync.dma_start(NEGID[:], negid[:])
            ID4 = cst.tile([128, 512], F16)
            nc.sync.dma_start(ID4[:], id4[:])

            QK = cst.tile([128, HW], F16)
            K2 = cst.tile([64, HW], F16)
            VRES = cst.tile([128, 128 * VBW], BF16)
            VB = cst.tile([128, 12 * VBW], BF16)
            SR = cst.tile([128, 128], F32)
            SC = cst.tile([128, 128], F32)
            BIASC = cst.tile([128, 1], F32)
            nc.vector.memset(BIASC[:], -DELTA)
            vres_view = VRES[:].rearrange("p (s w) -> p s w", w=VBW)
            vb_view = VB[:].rearrange("p (s w) -> p s w", w=VBW)
            nc.vector.memset(vres_view[:, :, 256:258], 1.0)
            nc.vector.memset(vb_view[:, :, 256:258], 1.0)

            qk_of = QK[0:64, :].rearrange("c (y x) -> c y x", x=128)
            k2_of = K2[:, :].rearrange("c (y x) -> c y x", x=128)

            # =================== row pass ===================
            state = {}
            state[("xs", 0)] = xpre[0]
            state[("xs", 1)] = xpre[1]

            def load_x(ch):
                xt = xsp.tile([128, 2048], F16, tag="xs")
                nc.sync.dma_start(xt[:], xin[ch])
                xbt = None
                if with_bias:
                    xbt = xsp.tile([2, 2048], F16, tag="xb")
                    nc.sync.dma_start(xbt[:], xbias[ch])
                state[("xs", ch)] = (xt, xbt)

            def mm_in(xs, k, sl):
                xt, xbt = xs
                if k < 4:
                    return xt[:, k * 512:k * 512 + 512][:, sl]
                return xbt[:, sl]

            def row_head(i):
                xs = state.pop(("xs", i))
                csl = slice(i * 512, (i + 1) * 512)
                pqk = psbigp.tile([128, 512], F32, tag="psbig")
                for k in range(nck):
                    rows = 128 if k < 4 else 2
                    nc.tensor.matmul(pqk[:],
                                     WQK[0:rows, k * 128:(k + 1) * 128],
                                     mm_in(xs, k, slice(0, 512)),
                                     start=(k == 0), stop=(k == nck - 1))
                nc.scalar.activation(QK[:, csl], pqk[:], COPY)
                nc.gpsimd.tensor_copy(K2[:, csl], QK[64:128, csl])
                idx = i % 3
                for hv in range(2):       # two [128,512] pv tiles: yy pairs
                    pv = psvp.tile([128, 512], F32, tag="psv")
                    for q2 in range(2):
                        yy = hv * 2 + q2
                        xsl = slice(yy * 128, (yy + 1) * 128)
                        for k in range(nck):
                            rows = 128 if k < 4 else 2
                            nc.tensor.matmul(
                                pv[:, q2 * 256:(q2 + 1) * 256],
                                mm_in(xs, k, xsl),
                                WV[0:rows, k * CV:(k + 1) * CV],
                                start=(k == 0), stop=(k == nck - 1))
                    # evac both yy slots in one strided DVE copy
                    nc.vector.tensor_copy(
                        vb_view[:, idx * 4 + hv * 2:idx * 4 + hv * 2 + 2,
                                0:256],
                        pv[:].rearrange("p (s w) -> p s w", w=256))

            def row_mid(j):
                pE = psbigp.tile([128, 512], F32, tag="psbig")
                for yy in range(4):
                    y = j * 4 + yy
                    ysl = slice(y * 128, (y + 1) * 128)
                    nc.tensor.matmul(pE[:, yy * 128:(yy + 1) * 128],
                                     K2[:, ysl], QK[0:64, ysl],
                                     start=True, stop=True)
                p4 = p4p.tile([128, 512], BF16, tag="p4")
                nc.scalar.activation(p4[:], pE[:], EXP, bias=BIASC[:])
                state[("p4", j)] = p4

            def row_tail(j):
                idx = j % 3
                p4 = state.pop(("p4", j))
                o16 = o16rp.tile([128, 1024], BF16, tag="o16r")
                for half in range(2):
                    pO = psOp.tile([128, 1024], F32, tag="psO")
                    for q2 in range(2):
                        yy = half * 2 + q2
                        nc.tensor.matmul(
                            pO[:, q2 * 512:q2 * 512 + VBW],
                            p4[:, yy * 128:(yy + 1) * 128],
                            VB[:, (idx * 4 + yy) * VBW:
                               (idx * 4 + yy + 1) * VBW],
                            start=True, stop=True)
                    y0 = j * 4 + half * 2
                    nc.vector.tensor_copy(
                        SR[:, y0:y0 + 2],
                        pO[:].rearrange("p (b k) -> p b k", k=512)[:, :, 256])
                    src = pO[:].rearrange(
                        "p (b k) -> p b k", k=512)[:, :, 0:256]
                    dst = o16[:, half * 512:(half + 1) * 512].rearrange(
                        "p (b k) -> p b k", k=256)
                    if half == 0:
                        nc.scalar.activation(dst, src, COPY)
                    else:
                        nc.vector.tensor_copy(dst, src)
                # orow chunk j: dst [x, 4 y, 256] contiguous per partition
                nc.sync.dma_start(
                    orow[:, j * 4:(j + 1) * 4, :],
                    o16[:].rearrange("p (t c) -> p t c", c=256))
                # scatter VB slots into column-resident VRES (4 1-part DMAs)
                for yy in range(4):
                    y = j * 4 + yy
                    dma_eng = nc.sync if yy < 2 else nc.scalar
                    dma_eng.dma_start(
                        VRES[y:y + 1, :].rearrange(
                            "p (s w) -> p s w", w=VBW)[:, :, 0:256],
                        vb_view[:, idx * 4 + yy, 0:256])

            for i in range(NCH + 2):
                if i < NCH:
                    if i + 2 < NCH:
                        load_x(i + 2)
                    row_head(i)
                if 1 <= i < NCH + 1:
                    row_mid(i - 1)
                if i >= 2:
                    row_tail(i - 2)

            # =================== column pass ===================
            cstate = {}

            def col_head(g):
                p4s = []
                for gg in range(2):
                    pE = psbigp.tile([128, 512], F32, tag="psbig")
                    for xx in range(4):
                        x = g * 8 + gg * 4 + xx
                        nc.tensor.matmul(pE[:, xx * 128:(xx + 1) * 128],
                                         k2_of[:, :, x], qk_of[:, :, x],
                                         start=(xx == 0), stop=False)
                    nc.tensor.matmul(pE[:], NEGID[:], ID4[:],
                                     start=False, stop=True)
                    p4 = p4cp.tile([128, 512], BF16, tag="p4c")
                    nc.scalar.activation(p4[:], pE[:], EXP, bias=BIASC[:])
                    p4s.append(p4)
                cstate[g] = p4s

            def col_tail(g):
                p4s = cstate.pop(g)
                o16 = o16cp.tile([128, 2048], BF16, tag="o16c")
                for gg in range(2):
                    p4 = p4s[gg]
                    for half in range(2):
                        pO = psOp.tile([128, 1024], F32, tag="psO")
                        for q2 in range(2):
                            xx = half * 2 + q2
                            x = g * 8 + gg * 4 + xx
                            nc.tensor.matmul(
                                pO[:, q2 * 512:q2 * 512 + VBW],
                                p4[:, xx * 128:(xx + 1) * 128],
                                VRES[:, x * VBW:(x + 1) * VBW],
                                start=True, stop=True)
                        x0 = g * 8 + gg * 4 + half * 2
                        nc.vector.tensor_copy(
                            SC[:, x0:x0 + 2],
                            pO[:].rearrange("p (b k) -> p b k",
                                            k=512)[:, :, 256])
                        src = pO[:].rearrange(
                            "p (b k) -> p b k", k=512)[:, :, 0:256]
                        oco = (gg * 2 + half) * 512
                        dst = o16[:, oco:oco + 512].rearrange(
                            "p (b k) -> p b k", k=256)
                        if half == 0:
                            nc.scalar.activation(dst, src, COPY)
                        else:
                            nc.vector.tensor_copy(dst, src)
                nc.sync.dma_start(
                    ocol[:, g * 8:(g + 1) * 8, :],
                    o16[:].rearrange("p (t c) -> p t c", c=256))

            for g in range(NSCH + 1):
                if g < NSCH:
                    col_head(g)
                if g >= 1:
                    col_tail(g - 1)

            nc.sync.dma_start(ssr[:], SR[:])
            nc.sync.dma_start(ssc[:], SC[:])

    nc.compile()
    return nc


def _get_nc(with_bias):
    key = bool(with_bias)
    if key not in _CACHE:
        _CACHE[key] = _build(key)
    return _CACHE[key]


def kernel(x, Wq, bq, Wk, bk, Wv, bv, _trace=False, _raw=False):
    x = np.asarray(x, np.float32)
    Wq = np.asarray(Wq, np.float32)
    Wk = np.asarray(Wk, np.float32)
    Wv = np.asarray(Wv, np.float32)
    bq = np.asarray(bq, np.float32)
    bk = np.asarray(bk, np.float32)
    bv = np.asarray(bv, np.float32)

    with_bias = bool(np.any(bq) or np.any(bk) or np.any(bv))
    nc = _get_nc(with_bias)

    negid_a = (-60000.0 * np.eye(128)).astype(np.float16)
    id4_a = np.tile(np.eye(128), (1, 4)).astype(np.float16)
    wqk_full = np.concatenate([Wq.T, Wk.T], axis=1)       # [C, 128]
    if with_bias:
        bias_qk = np.concatenate([bq, bk])[None, :]
        wqk_full = np.concatenate(
            [wqk_full, bias_qk, np.zeros_like(bias_qk)], axis=0)
    wqk_full = wqk_full.astype(np.float16)

    in_maps = []
    for core in range(N_CORES):
        b, h = core // 2, core % 2
        # xin[ch, p, k*512+w] = x[b, 128k+p, ch*512+w]
        xb = np.ascontiguousarray(
            x[b].reshape(4, 128, NCH, 512).transpose(2, 1, 0, 3)
            .reshape(NCH, 128, 2048)).astype(np.float16)
        wvh = Wv[h * CV:(h + 1) * CV, :].T                # [C, CV]
        if with_bias:
            bvh = bv[h * CV:(h + 1) * CV][None, :]
            wvh = np.concatenate([wvh, bvh, np.zeros_like(bvh)], axis=0)
        m = {
            "xin": xb, "wqk": wqk_full,
            "wv": wvh.astype(np.float16),
            "negid": negid_a, "id4": id4_a,
        }
        if with_bias:
            ob = np.zeros((NCH, 2, 2048), np.float32)
            ob[:, 0, :] = 1.0
            m["xbias"] = ob.astype(np.float16)
        in_maps.append(m)

    res = run_bass_kernel_spmd(nc, in_maps, list(range(N_CORES)),
                               trace=bool(_trace))
    if _raw:
        return res

    out = np.empty((B, C, H, W), np.float32)
    for core in range(N_CORES):
        b, h = core // 2, core % 2
        r = res.results[core]
        o_r = r["orow"].astype(np.float32)     # [x, y, c] unnormalized
        o_c = r["ocol"].astype(np.float32)     # [y, x, c] unnormalized
        s_r = r["ssr"].T                       # [y, x] = S_row
        s_c = r["ssc"]                         # [y, x] = S_col
        g = 1.0 / (s_r + s_c)
        comb = (o_r.transpose(1, 0, 2) + o_c) * g[:, :, None]  # [y, x, c]
        out[b, h * CV:(h + 1) * CV] = comb.transpose(2, 0, 1)

    if _trace:
        return out, res
    return out


# revision 11
# speedup vs baseline: 1.6440x; 1.6440x over previous
"""Criss-cross (CCNet) attention kernel for Trainium2, 8 NeuronCores.

Sharding: core c in 0..7 -> batch b = c//2, value-channel half h = c%2.
Each core computes, for its (b, h), the full joint row+column softmax
attention over 256 of the 512 value/output channels.

Design (v2, fp16 pipeline):
  - All matmuls are fp16 (1 cyc/row at any moving size, vs fp32r's 4x
    penalty under 256), with fp32 PSUM accumulation.
  - Energies E = k.q are shifted by a global constant DELTA before exp;
    P = exp(E-DELTA) is stored in bf16 (wide exponent range covers the
    ~33-nat spread of per-row maxima; fp16 cannot).  The shift cancels
    exactly in the final combine since both passes share it.
  - Outputs are UN-normalized:  orow = P^T V  in bf16 (wide exponent
    range), plus the row/col sums S (fp32).  The host computes
    out = (o_row + o_col^T) / (S_row + S_col), which equals the exact
    softmax combine; no on-device division or scaling at all.
  - V is staged through DRAM x-major as vscr[x][j][c] (one 512B-
    descriptor write per row chunk, one strided 512B-descriptor gather
    per column super-chunk); the partition transpose rides the DRAM
    addressing, never a slow single-partition SBUF DMA.
  - Deep software pipeline: iteration i runs projections(i),
    energies(i-1), aggregation+evac(i-2), so the tensor engine queue
    never waits on scalar/vector evacuations and holds its max p-state.
"""

import numpy as np

import concourse.tile as tile
from concourse import bacc, mybir
from concourse.bass_utils import run_bass_kernel_spmd

B, C, H, W = 4, 512, 128, 128
CQK = C // 8          # 64
CV = C // 2           # 256 v channels per core
HW = H * W
N_CORES = 8
NCH = 32              # row chunks (4 rows each)
NSCH = 16             # col super-chunks (8 cols each)
DELTA = 41.0          # exp shift (max energy on this data ~50.7)
VBW = 258             # v slot width: 256 channels + 2 ones columns

F32 = mybir.dt.float32
F16 = mybir.dt.float16
BF16 = mybir.dt.bfloat16
EXP = mybir.ActivationFunctionType.Exp
COPY = mybir.ActivationFunctionType.Copy

_CACHE = {}


def _build(with_bias):
    nc = bacc.Bacc("TRN2", target_bir_lowering=False, debug=False,
                   num_devices=N_CORES)
    nck = 5 if with_bias else 4    # contraction chunks (last = 2 bias rows)

    xin = nc.dram_tensor("xin", [NCH, 128, 2048], F16,
                         kind="ExternalInput").ap()
    xbias = nc.dram_tensor("xbias", [NCH, 2, 2048], F16,
                           kind="ExternalInput").ap() if with_bias else None
    wqk = nc.dram_tensor("wqk", [C + (2 if with_bias else 0), 128], F16,
                         kind="ExternalInput").ap()
    wv = nc.dram_tensor("wv", [C + (2 if with_bias else 0), CV], F16,
                        kind="ExternalInput").ap()
    negid = nc.dram_tensor("negid", [128, 128], F16,
                           kind="ExternalInput").ap()
    id4 = nc.dram_tensor("id4", [128, 512], F16, kind="ExternalInput").ap()

    vscr = nc.dram_tensor("vscr", [W, H, CV], BF16).ap()   # [x, j, c]
    # orow laid out [x, y, c]; ocol laid out [y, x, c]  (contiguous writes)
    orow = nc.dram_tensor("orow", [W, H, CV], BF16,
                          kind="ExternalOutput").ap()
    ocol = nc.dram_tensor("ocol", [H, W, CV], BF16,
                          kind="ExternalOutput").ap()
    ssr = nc.dram_tensor("ssr", [128, 128], F32, kind="ExternalOutput").ap()
    ssc = nc.dram_tensor("ssc", [128, 128], F32, kind="ExternalOutput").ap()

    with tile.TileContext(nc) as tc:
        with (
            tc.tile_pool(name="cst", bufs=1) as cst,
            tc.tile_pool(name="xs", bufs=3) as xsp,
            tc.tile_pool(name="p4", bufs=3) as p4p,
            tc.tile_pool(name="p4c", bufs=4) as p4cp,
            tc.tile_pool(name="o16r", bufs=2) as o16rp,
            tc.tile_pool(name="o16c", bufs=2) as o16cp,
            tc.tile_pool(name="psbig", bufs=2, space="PSUM") as psbigp,
            tc.tile_pool(name="psv", bufs=2, space="PSUM") as psvp,
            tc.tile_pool(name="psO", bufs=2, space="PSUM") as psOp,
        ):
            # ---- prefetch x chunks 0,1 before everything ----
            xpre = {}
            for ch0 in range(2):
                xt = xsp.tile([128, 2048], F16, tag="xs", name=f"xpre{ch0}")
                nc.sync.dma_start(xt[:], xin[ch0])
                if with_bias:
                    xbt = xsp.tile([2, 2048], F16, tag="xb",
                                   name=f"xbpre{ch0}")
                    nc.sync.dma_start(xbt[:], xbias[ch0])
                    xpre[ch0] = (xt, xbt)
                else:
                    xpre[ch0] = (xt, None)

            # ---- persistent constants ----
            WQK = cst.tile([128, nck * 128], F16)
            for k in range(nck):
                rows = 128 if k < 4 else 2
                nc.sync.dma_start(WQK[0:rows, k * 128:(k + 1) * 128],
                                  wqk[k * 128:k * 128 + rows, :])
            WV = cst.tile([128, nck * CV], F16)
            for k in range(nck):
                rows = 128 if k < 4 else 2
                nc.sync.dma_start(WV[0:rows, k * CV:(k + 1) * CV],
                                  wv[k * 128:k * 128 + rows, :])
            NEGID = cst.tile([128, 128], F16)
            nc.sync.dma_start(NEGID[:], negid[:])
            ID4 = cst.tile([128, 512], F16)
            nc.sync.dma_start(ID4[:], id4[:])

            QK = cst.tile([128, HW], F16)
            K2 = cst.tile([64, HW], F16)
            VTB = cst.tile([128, 16 * VBW], BF16)
            VB = cst.tile([128, 12 * VBW], BF16)
            SR = cst.tile([128, 128], F32)
            SC = cst.tile([128, 128], F32)
            BIASC = cst.tile([128, 1], F32)
            nc.vector.memset(BIASC[:], -DELTA)
            vtb_view = VTB[:].rearrange("p (s w) -> p s w", w=VBW)
            vb_view = VB[:].rearrange("p (s w) -> p s w", w=VBW)
            nc.vector.memset(vtb_view[:, :, 256:258], 1.0)
            nc.vector.memset(vb_view[:, :, 256:258], 1.0)


            qk_of = QK[0:64, :].rearrange("c (y x) -> c y x", x=128)
            k2_of = K2[:, :].rearrange("c (y x) -> c y x", x=128)

            # =================== row pass ===================
            state = {}
            state[("xs", 0)] = xpre[0]
            state[("xs", 1)] = xpre[1]

            def load_x(ch):
                xt = xsp.tile([128, 2048], F16, tag="xs")
                nc.sync.dma_start(xt[:], xin[ch])
                xbt = None
                if with_bias:
                    xbt = xsp.tile([2, 2048], F16, tag="xb")
                    nc.sync.dma_start(xbt[:], xbias[ch])
                state[("xs", ch)] = (xt, xbt)

            def mm_in(xs, k, sl):
                xt, xbt = xs
                if k < 4:
                    return xt[:, k * 512:k * 512 + 512][:, sl]
                return xbt[:, sl]

            def row_head(i):
                xs = state.pop(("xs", i))
                csl = slice(i * 512, (i + 1) * 512)
                pqk = psbigp.tile([128, 512], F32, tag="psbig")
                for k in range(nck):
                    rows = 128 if k < 4 else 2
                    nc.tensor.matmul(pqk[:],
                                     WQK[0:rows, k * 128:(k + 1) * 128],
                                     mm_in(xs, k, slice(0, 512)),
                                     start=(k == 0), stop=(k == nck - 1))
                nc.scalar.activation(QK[:, csl], pqk[:], COPY)
                nc.gpsimd.tensor_copy(K2[:, csl], QK[64:128, csl])
                idx = i % 3
                for hv in range(2):       # two [128,512] pv tiles: yy pairs
                    pv = psvp.tile([128, 512], F32, tag="psv")
                    for q2 in range(2):
                        yy = hv * 2 + q2
                        xsl = slice(yy * 128, (yy + 1) * 128)
                        for k in range(nck):
                            rows = 128 if k < 4 else 2
                            nc.tensor.matmul(
                                pv[:, q2 * 256:(q2 + 1) * 256],
                                mm_in(xs, k, xsl),
                                WV[0:rows, k * CV:(k + 1) * CV],
                                start=(k == 0), stop=(k == nck - 1))
                    # evac both yy slots in one strided DVE copy
                    nc.vector.tensor_copy(
                        vb_view[:, idx * 4 + hv * 2:idx * 4 + hv * 2 + 2,
                                0:256],
                        pv[:].rearrange("p (s w) -> p s w", w=256))

            def row_mid(j):
                pE = psbigp.tile([128, 512], F32, tag="psbig")
                for yy in range(4):
                    y = j * 4 + yy
                    ysl = slice(y * 128, (y + 1) * 128)
                    nc.tensor.matmul(pE[:, yy * 128:(yy + 1) * 128],
                                     K2[:, ysl], QK[0:64, ysl],
                                     start=True, stop=True)
                p4 = p4p.tile([128, 512], BF16, tag="p4")
                nc.scalar.activation(p4[:], pE[:], EXP, bias=BIASC[:])
                state[("p4", j)] = p4

            def row_tail(j):
                idx = j % 3
                p4 = state.pop(("p4", j))
                o16 = o16rp.tile([128, 1024], BF16, tag="o16r")
                for half in range(2):
                    pO = psOp.tile([128, 1024], F32, tag="psO")
                    for q2 in range(2):
                        yy = half * 2 + q2
                        nc.tensor.matmul(
                            pO[:, q2 * 512:q2 * 512 + VBW],
                            p4[:, yy * 128:(yy + 1) * 128],
                            VB[:, (idx * 4 + yy) * VBW:
                               (idx * 4 + yy + 1) * VBW],
                            start=True, stop=True)
                    y0 = j * 4 + half * 2
                    nc.vector.tensor_copy(
                        SR[:, y0:y0 + 2],
                        pO[:].rearrange("p (b k) -> p b k", k=512)[:, :, 256])
                    src = pO[:].rearrange(
                        "p (b k) -> p b k", k=512)[:, :, 0:256]
                    dst = o16[:, half * 512:(half + 1) * 512].rearrange(
                        "p (b k) -> p b k", k=256)
                    if half == 0:
                        nc.scalar.activation(dst, src, COPY)
                    else:
                        nc.vector.tensor_copy(dst, src)
                # orow chunk j: dst [x, 4 y, 256] contiguous per partition
                nc.sync.dma_start(
                    orow[:, j * 4:(j + 1) * 4, :],
                    o16[:].rearrange("p (t c) -> p t c", c=256))
                # stage VB slots to DRAM x-major: dst [x, 4 y, c] matches
                # the VB source order (x-part, yy, c); 512B descriptors
                nc.sync.dma_start(
                    vscr[:, j * 4:(j + 1) * 4, :],
                    vb_view[:, idx * 4:idx * 4 + 4, 0:256])

            for i in range(NCH + 2):
                if i < NCH:
                    if i + 2 < NCH:
                        load_x(i + 2)
                    row_head(i)
                if 1 <= i < NCH + 1:
                    row_mid(i - 1)
                if i >= 2:
                    row_tail(i - 2)

            # =================== column pass ===================
            cstate = {}

            def col_head(g):
                # prefetch this super-chunk's column-gathered v (used in
                # col_tail(g) one iteration later)
                nc.sync.dma_start(
                    vtb_view[:, (g % 2) * 8:(g % 2) * 8 + 8, 0:256],
                    vscr[g * 8:(g + 1) * 8, :, :].rearrange(
                        "t j c -> j t c"))
                p4s = []
                for gg in range(2):
                    pE = psbigp.tile([128, 512], F32, tag="psbig")
                    for xx in range(4):
                        x = g * 8 + gg * 4 + xx
                        nc.tensor.matmul(pE[:, xx * 128:(xx + 1) * 128],
                                         k2_of[:, :, x], qk_of[:, :, x],
                                         start=(xx == 0), stop=False)
                    nc.tensor.matmul(pE[:], NEGID[:], ID4[:],
                                     start=False, stop=True)
                    p4 = p4cp.tile([128, 512], BF16, tag="p4c")
                    nc.scalar.activation(p4[:], pE[:], EXP, bias=BIASC[:])
                    p4s.append(p4)
                cstate[g] = p4s

            def col_tail(g):
                p4s = cstate.pop(g)
                o16 = o16cp.tile([128, 2048], BF16, tag="o16c")
                for gg in range(2):
                    p4 = p4s[gg]
                    for half in range(2):
                        pO = psOp.tile([128, 1024], F32, tag="psO")
                        for q2 in range(2):
                            xx = half * 2 + q2
                            x = g * 8 + gg * 4 + xx
                            slot = (g % 2) * 8 + gg * 4 + xx
                            nc.tensor.matmul(
                                pO[:, q2 * 512:q2 * 512 + VBW],
                                p4[:, xx * 128:(xx + 1) * 128],
                                VTB[:, slot * VBW:(slot + 1) * VBW],
                                start=True, stop=True)
                        x0 = g * 8 + gg * 4 + half * 2
                        nc.vector.tensor_copy(
                            SC[:, x0:x0 + 2],
                            pO[:].rearrange("p (b k) -> p b k",
                                            k=512)[:, :, 256])
                        src = pO[:].rearrange(
                            "p (b k) -> p b k", k=512)[:, :, 0:256]
                        oco = (gg * 2 + half) * 512
                        dst = o16[:, oco:oco + 512].rearrange(
                            "p (b k) -> p b k", k=256)
                        if half == 0:
                            nc.scalar.activation(dst, src, COPY)
                        else:
                            nc.vector.tensor_copy(dst, src)
                nc.sync.dma_start(
                    ocol[:, g * 8:(g + 1) * 8, :],
                    o16[:].rearrange("p (t c) -> p t c", c=256))

            for g in range(NSCH + 1):
                if g < NSCH:
                    col_head(g)
                if g >= 1:
                    col_tail(g - 1)

            nc.sync.dma_start(ssr[:], SR[:])
            nc.sync.dma_start(ssc[:], SC[:])

    nc.compile()
    return nc


def _get_nc(with_bias):
    key = bool(with_bias)
    if key not in _CACHE:
        _CACHE[key] = _build(key)
    return _CACHE[key]


def kernel(x, Wq, bq, Wk, bk, Wv, bv, _trace=False, _raw=False):
    x = np.asarray(x, np.float32)
    Wq = np.asarray(Wq, np.float32)
    Wk = np.asarray(Wk, np.float32)
    Wv = np.asarray(Wv, np.float32)
    bq = np.asarray(bq, np.float32)
    bk = np.asarray(bk, np.float32)
    bv = np.asarray(bv, np.float32)

    with_bias = bool(np.any(bq) or np.any(bk) or np.any(bv))
    nc = _get_nc(with_bias)

    negid_a = (-60000.0 * np.eye(128)).astype(np.float16)
    id4_a = np.tile(np.eye(128), (1, 4)).astype(np.float16)
    wqk_full = np.concatenate([Wq.T, Wk.T], axis=1)       # [C, 128]
    if with_bias:
        bias_qk = np.concatenate([bq, bk])[None, :]
        wqk_full = np.concatenate(
            [wqk_full, bias_qk, np.zeros_like(bias_qk)], axis=0)
    wqk_full = wqk_full.astype(np.float16)

    in_maps = []
    for core in range(N_CORES):
        b, h = core // 2, core % 2
        # xin[ch, p, k*512+w] = x[b, 128k+p, ch*512+w]
        xb = np.ascontiguousarray(
            x[b].reshape(4, 128, NCH, 512).transpose(2, 1, 0, 3)
            .reshape(NCH, 128, 2048)).astype(np.float16)
        wvh = Wv[h * CV:(h + 1) * CV, :].T                # [C, CV]
        if with_bias:
            bvh = bv[h * CV:(h + 1) * CV][None, :]
            wvh = np.concatenate([wvh, bvh, np.zeros_like(bvh)], axis=0)
        m = {
            "xin": xb, "wqk": wqk_full,
            "wv": wvh.astype(np.float16),
            "negid": negid_a, "id4": id4_a,
        }
        if with_bias:
            ob = np.zeros((NCH, 2, 2048), np.float32)
            ob[:, 0, :] = 1.0
            m["xbias"] = ob.astype(np.float16)
        in_maps.append(m)

    res = run_bass_kernel_spmd(nc, in_maps, list(range(N_CORES)),
                               trace=bool(_trace))
    if _raw:
        return res

    out = np.empty((B, C, H, W), np.float32)
    for core in range(N_CORES):
        b, h = core // 2, core % 2
        r = res.results[core]
        o_r = r["orow"].astype(np.float32)     # [x, y, c] unnormalized
        o_c = r["ocol"].astype(np.float32)     # [y, x, c] unnormalized
        s_r = r["ssr"].T                       # [y, x] = S_row
        s_c = r["ssc"]                         # [y, x] = S_col
        g = 1.0 / (s_r + s_c)
        comb = (o_r.transpose(1, 0, 2) + o_c) * g[:, :, None]  # [y, x, c]
        out[b, h * CV:(h + 1) * CV] = comb.transpose(2, 0, 1)

    if _trace:
        return out, res
    return out
